# revision 33
# baseline (speedup 1.0000x reference)
"""Trainium2 Bass kernel for nn_BlockWithAdapter (B=2, T=2048, C=1024, H=16, M=64).

Strategy v2: head-parallel attention. Core c = (batch b=c//4, head-group
g=c%4, heads 4g..4g+3). Each core computes LN1 + Q/K/V for the FULL 2048-token
sequence of its batch but only its 4 heads, then causal attention with
uniform trip counts (query-quarter i attends to key blocks 0..4i+3 — same
count on every core, so the SPMD program stays uniform while skipping ~38%
of the score/AV work). Attention outputs y^T (plus reciprocal softmax
denominators) are exchanged with ONE small AllToAll (532KB/core) that
re-shards from head-parallel to token-parallel; proj and the whole back half
(adapter1, LN2, MLP, adapter2) then run token-local on the core's own 512
tokens exactly as in v1. No K/V AllGathers, no big collectives on the
critical path. A tiny dummy AllGather at kernel start absorbs the one-time
CC init barrier off the critical path. All matmuls bf16 (N=512 moving).
"""
import sys
import types

sys.path.insert(0, '/opt/trn_rl_repo')

import ml_dtypes
import numpy as np

import concourse.bass as bass  # noqa: F401  (registers arch)
import concourse.mybir as mybir
import concourse.tile as tile
from concourse import bacc
from concourse import bass_utils

P = 128
B, T, C, H, M = 2, 2048, 1024, 16, 64
HD = C // H            # 64
R = 512                # tokens per core in the token-parallel phases
CT = C // P            # 8 feature tiles of the residual stream
NCORES = 8
EPS = 1e-5
GRP = [[0, 1, 2, 3], [4, 5, 6, 7]]
GRP8 = [[0, 1, 2, 3, 4, 5, 6, 7]]
NCH = T // R           # 4 token chunks of the full sequence

FT = mybir.dt.float32
BF = mybir.dt.bfloat16
AF = mybir.ActivationFunctionType
OP = mybir.AluOpType

_CACHE = {}


def _build(stage='full'):
    nc = bacc.Bacc("TRN2", target_bir_lowering=False, debug=False,
                   num_devices=NCORES)

    # ---- DRAM I/O ----
    d_xf = nc.dram_tensor("xf", [P, CT, T], BF, kind="ExternalInput")
    d_xT = nc.dram_tensor("xT", [P, CT, R], FT, kind="ExternalInput")
    d_mask = nc.dram_tensor("mask", [P, 4, 2 * R], BF, kind="ExternalInput")
    d_wqk = nc.dram_tensor("wqk", [4, P, CT, P], BF, kind="ExternalInput")
    d_wv = nc.dram_tensor("wv", [P, CT, 256], BF, kind="ExternalInput")
    d_bqk = nc.dram_tensor("bqk", [P, 4], FT, kind="ExternalInput")
    d_bv = nc.dram_tensor("bv", [1, 256], BF, kind="ExternalInput")
    d_wproj = nc.dram_tensor("wproj", [CT, P, CT, P], BF, kind="ExternalInput")
    d_wfc = nc.dram_tensor("wfc", [32, P, CT, P], BF, kind="ExternalInput")
    d_wmp = nc.dram_tensor("wmp", [CT, P, 32, P], BF, kind="ExternalInput")
    d_a1d = nc.dram_tensor("a1d", [P, CT, M], BF, kind="ExternalInput")
    d_a1u = nc.dram_tensor("a1u", [M, CT, P], BF, kind="ExternalInput")
    d_a2d = nc.dram_tensor("a2d", [P, CT, M], BF, kind="ExternalInput")
    d_a2u = nc.dram_tensor("a2u", [M, CT, P], BF, kind="ExternalInput")
    d_bproj = nc.dram_tensor("bproj", [P, CT], FT, kind="ExternalInput")
    d_bfc = nc.dram_tensor("bfc", [P, 32], FT, kind="ExternalInput")
    d_bmp = nc.dram_tensor("bmp", [P, CT], FT, kind="ExternalInput")
    d_b1d = nc.dram_tensor("b1d", [M, 1], FT, kind="ExternalInput")
    d_b1u = nc.dram_tensor("b1u", [P, CT], FT, kind="ExternalInput")
    d_b2d = nc.dram_tensor("b2d", [M, 1], FT, kind="ExternalInput")
    d_b2u = nc.dram_tensor("b2u", [P, CT], FT, kind="ExternalInput")
    d_ln1g = nc.dram_tensor("ln1g", [P, CT], FT, kind="ExternalInput")
    d_ln1b = nc.dram_tensor("ln1b", [P, CT], FT, kind="ExternalInput")
    d_ln2g = nc.dram_tensor("ln2g", [P, CT], FT, kind="ExternalInput")
    d_ln2b = nc.dram_tensor("ln2b", [P, CT], FT, kind="ExternalInput")
    d_zc = nc.dram_tensor("zc", [16, 2], FT, kind="ExternalInput")
    d_ident = nc.dram_tensor("ident", [P, P], BF, kind="ExternalInput")
    d_ones = nc.dram_tensor("onesc", [P, 1], BF, kind="ExternalInput")
    d_onesr = nc.dram_tensor("onesr", [1, P], BF, kind="ExternalInput")
    d_vones = nc.dram_tensor("vones", [P, 16], BF, kind="ExternalInput")
    d_sel16 = nc.dram_tensor("sel16", [CT, 16, P], BF, kind="ExternalInput")
    d_out = nc.dram_tensor("out", [CT, P, R], FT, kind="ExternalOutput")

    STAGES = ['x', 'ln1', 'qkv', 'attn', 'proj', 'a1', 'mlp', 'full']
    s_idx = STAGES.index(stage)

    def want(s):
        return s_idx >= STAGES.index(s)

    with tile.TileContext(nc) as tc, \
         nc.allow_low_precision(reason="bf16 matmuls within tolerance"):
        with tc.tile_pool(name="const", bufs=1) as cp, \
             tc.tile_pool(name="resid", bufs=1) as rp, \
             tc.tile_pool(name="psum", bufs=2, space="PSUM") as pp, \
             tc.tile_pool(name="lnsb", bufs=4) as lp, \
             tc.tile_pool(name="sq", bufs=3) as sqp, \
             tc.tile_pool(name="dram", bufs=1, space="DRAM") as dp:

            # ---- constants (gpsimd DMA queue, off the xf critical path) ----
            ones = cp.tile([P, 1], BF, name="ones")
            nc.gpsimd.dma_start(ones[:], d_ones[:])
            onesr = cp.tile([1, P], BF, name="onesr")
            nc.gpsimd.dma_start(onesr[:], d_onesr[:])
            vones_sb = cp.tile([P, 16], BF, name="vones_sb")
            nc.gpsimd.dma_start(vones_sb[:], d_vones[:])
            sel16_sb = cp.tile([16, CT, P], BF, name="sel16_sb")
            nc.gpsimd.dma_start(sel16_sb[:], d_sel16[:].transpose([1, 0, 2]))
            eps_sb = cp.tile([1, 1], FT, name="eps_sb")
            nc.vector.memset(eps_sb[:], EPS)
            mask_sb = cp.tile([P, 4, 2 * R], BF, name="mask_sb")
            nc.gpsimd.dma_start(mask_sb[:], d_mask[:])
            zc_sb = cp.tile([16, 2], FT, name="zc_sb")
            nc.gpsimd.dma_start(zc_sb[:], d_zc[:])
            ident_sb = cp.tile([P, P], BF, name="ident_sb")
            nc.gpsimd.dma_start(ident_sb[:], d_ident[:])
            bqk_sb = cp.tile([P, 4], FT, name="bqk_sb")
            nc.gpsimd.dma_start(bqk_sb[:], d_bqk[:])
            bv_sb = cp.tile([1, 256], BF, name="bv_sb")
            nc.gpsimd.dma_start(bv_sb[:], d_bv[:])
            bproj_sb = cp.tile([P, CT], FT, name="bproj_sb")
            nc.gpsimd.dma_start(bproj_sb[:], d_bproj[:])
            bfc_sb = cp.tile([P, 32], FT, name="bfc_sb")
            nc.gpsimd.dma_start(bfc_sb[:], d_bfc[:])
            bmp_sb = cp.tile([P, CT], FT, name="bmp_sb")
            nc.gpsimd.dma_start(bmp_sb[:], d_bmp[:])
            b1d_sb = cp.tile([M, 1], FT, name="b1d_sb")
            nc.gpsimd.dma_start(b1d_sb[:], d_b1d[:])
            b1u_sb = cp.tile([P, CT], FT, name="b1u_sb")
            nc.gpsimd.dma_start(b1u_sb[:], d_b1u[:])
            b2d_sb = cp.tile([M, 1], FT, name="b2d_sb")
            nc.gpsimd.dma_start(b2d_sb[:], d_b2d[:])
            b2u_sb = cp.tile([P, CT], FT, name="b2u_sb")
            nc.gpsimd.dma_start(b2u_sb[:], d_b2u[:])
            ln1g_sb = cp.tile([P, CT], FT, name="ln1g_sb")
            nc.gpsimd.dma_start(ln1g_sb[:], d_ln1g[:])
            ln1b_sb = cp.tile([P, CT], FT, name="ln1b_sb")
            nc.gpsimd.dma_start(ln1b_sb[:], d_ln1b[:])
            ln2g_sb = cp.tile([P, CT], FT, name="ln2g_sb")
            nc.gpsimd.dma_start(ln2g_sb[:], d_ln2g[:])
            ln2b_sb = cp.tile([P, CT], FT, name="ln2b_sb")
            nc.gpsimd.dma_start(ln2b_sb[:], d_ln2b[:])

            # ---- dummy collective: absorb CC init/barrier early ----
            cc_din = dp.tile([1, 64], BF, name="cc_din")
            cc_dout = dp.tile([8, 64], BF, name="cc_dout")
            dumt = cp.tile([1, 64], BF, name="dumt")
            nc.vector.memset(dumt[:], 1.0)
            nc.sync.dma_start(cc_din[:], dumt[:])
            nc.gpsimd.collective_compute(
                "AllGather", OP.bypass, replica_groups=GRP8,
                ins=[cc_din[:]], outs=[cc_dout[:]])

            # ---- residual stream (own 512 tokens, transposed, fp32) ----
            X = []
            for ct in range(CT):
                xt = rp.tile([P, R], FT, name=f"x{ct}")
                nc.gpsimd.dma_start(xt[:], d_xT[:, ct, :])
                X.append(xt)

            def dump_ft(tiles):
                for i, t in enumerate(tiles[:CT]):
                    nc.sync.dma_start(d_out[i], t[:])

            def dump_bf(aps):
                # debug only: cast bf16 APs ([P, R]) to f32 and dump
                for i, a in enumerate(aps[:CT]):
                    tmp = lp.tile([P, R], FT, name=f"dump{i}", tag="dump",
                                  bufs=2)
                    nc.scalar.copy(tmp[:], a)
                    nc.sync.dma_start(d_out[i], tmp[:])

            # persistent QKV tensors (4 local heads, full T)
            with tc.tile_pool(name="qkT", bufs=1) as qkp, \
                 tc.tile_pool(name="vT", bufs=1) as vp:
                qT = [qkp.tile([P, T], BF, name=f"q{i}") for i in range(2)]
                kT = [qkp.tile([P, T], BF, name=f"k{i}") for i in range(2)]
                v_h = [vp.tile([P, 16, 130], BF, name=f"v{i}")
                       for i in range(2)]

                # ============ LN1 (full T, 4 chunks) + QKV ============
                with tc.tile_pool(name="xfp", bufs=1) as xfp, \
                     tc.tile_pool(name="xlnp", bufs=1) as xlp, \
                     tc.tile_pool(name="wqp", bufs=2) as wqp, \
                     tc.tile_pool(name="qps", bufs=4, space="PSUM") as qp4:
                    xf = [xfp.tile([P, T], BF, name=f"xf{ct}")
                          for ct in range(CT)]
                    # chunked loads so LN1 starts after the first MB
                    for ch in range(NCH):
                        sl = slice(R * ch, R * (ch + 1))
                        for ct in range(CT):
                            nc.sync.dma_start(xf[ct][:, sl],
                                              d_xf[:, ct, sl])
                    xln = [xlp.tile([P, T], BF, name=f"xln{ct}")
                           for ct in range(CT)]
                    # squares precomputed up front so chunk ch+1 stats never
                    # wait on chunk ch's normalize in the vector queue
                    xsqf = [xfp.tile([P, T], BF, name=f"xsqf{ct}")
                            for ct in range(CT)]
                    if want('ln1'):
                        for ch in range(NCH):
                            sl = slice(R * ch, R * (ch + 1))
                            for ct in range(CT):
                                nc.vector.tensor_mul(xsqf[ct][:, sl],
                                                     xf[ct][:, sl],
                                                     xf[ct][:, sl])

                    for ch in (range(NCH) if want('ln1') else []):
                        sl = slice(R * ch, R * (ch + 1))
                        s_ps = pp.tile([1, R], FT, name="s_ln1", tag="mm")
                        q_ps = pp.tile([1, R], FT, name="q_ln1", tag="mm")
                        for ct in range(CT):
                            nc.tensor.matmul(s_ps[:], ones[:], xf[ct][:, sl],
                                             start=(ct == 0),
                                             stop=(ct == CT - 1))
                            nc.tensor.matmul(q_ps[:], ones[:],
                                             xsqf[ct][:, sl],
                                             start=(ct == 0),
                                             stop=(ct == CT - 1))
                        mu = lp.tile([1, R], BF, name="mu1", tag="ln")
                        nc.scalar.mul(mu[:], s_ps[:], 1.0 / C)
                        ex2 = lp.tile([1, R], FT, name="ex21", tag="ln")
                        nc.scalar.mul(ex2[:], q_ps[:], 1.0 / C)
                        var = lp.tile([1, R], FT, name="var1", tag="ln")
                        nc.vector.tensor_mul(var[:], mu[:], mu[:])
                        nc.vector.tensor_sub(var[:], ex2[:], var[:])
                        std = lp.tile([1, R], FT, name="std1", tag="ln")
                        nc.scalar.activation(std[:], var[:], AF.Sqrt,
                                             bias=eps_sb[:])
                        rstd = lp.tile([1, R], BF, name="rstd1", tag="ln")
                        nc.vector.reciprocal(rstd[:], std[:])
                        mu_b = pp.tile([P, R], FT, name="mub1", tag="mm")
                        nc.tensor.matmul(mu_b[:], onesr[:], mu[:])
                        rstd_b = pp.tile([P, R], FT, name="rsb1", tag="mm")
                        nc.tensor.matmul(rstd_b[:], onesr[:], rstd[:])
                        # copy broadcasts to SBUF so the psum ring frees
                        # immediately and the next chunk's stats can run
                        mub_sb = lp.tile([P, R], BF, name="mub_sb",
                                         tag="lnb", bufs=4)
                        nc.scalar.copy(mub_sb[:], mu_b[:])
                        rsb_sb = lp.tile([P, R], BF, name="rsb_sb",
                                         tag="lnb", bufs=4)
                        nc.scalar.copy(rsb_sb[:], rstd_b[:])
                        for ct in range(CT):
                            dst = xln[ct][:, sl]
                            nc.vector.tensor_sub(dst, xf[ct][:, sl],
                                                 mub_sb[:])
                            nc.vector.tensor_mul(dst, dst, rsb_sb[:])
                            nc.scalar.activation(
                                dst, dst, AF.Identity,
                                scale=ln1g_sb[:, ct:ct + 1],
                                bias=ln1b_sb[:, ct:ct + 1])
                    if stage == 'ln1':
                        dump_bf([xln[ct][:, 0:R] for ct in range(CT)])

                    # Q (m-tiles 0,1; pre-scaled 1/8 host-side) and K (2,3)
                    for mt in (range(4) if want('qkv') else []):
                        wt = wqp.tile([P, CT, P], BF, name="wqk_t", tag="wqk")
                        nc.sync.dma_start(wt[:], d_wqk[mt])
                        for ch in range(NCH):
                            sl = slice(R * ch, R * (ch + 1))
                            ps = qp4.tile([P, R], FT, name="ps_qk", tag="qk")
                            for kt in range(CT):
                                nc.tensor.matmul(ps[:], wt[:, kt, :],
                                                 xln[kt][:, sl],
                                                 start=(kt == 0),
                                                 stop=(kt == CT - 1))
                            dst = (qT[mt] if mt < 2 else kT[mt - 2])[:, sl]
                            nc.scalar.activation(dst, ps[:], AF.Identity,
                                                 bias=bqk_sb[:, mt:mt + 1])

                    # V in AV-ready layout [key, token-block, head-feat]
                    if want('qkv'):
                        wv_sb = wqp.tile([P, CT, 256], BF, name="wv_sb",
                                         tag="wv", bufs=1)
                        nc.sync.dma_start(wv_sb[:], d_wv[:])
                        for hp in range(2):
                            nc.sync.dma_start(v_h[hp][:, :, 64:65],
                                              vones_sb[:].unsqueeze(2))
                            nc.sync.dma_start(v_h[hp][:, :, 129:130],
                                              vones_sb[:].unsqueeze(2))
                        for tb in range(16):
                            ps = qp4.tile([P, 256], FT, name="ps_v",
                                          tag="qk")
                            for kt in range(CT):
                                nc.tensor.matmul(
                                    ps[:], xln[kt][:, P * tb:P * (tb + 1)],
                                    wv_sb[:, kt, :],
                                    start=(kt == 0), stop=False)
                            nc.tensor.matmul(ps[:], onesr[:], bv_sb[:],
                                             start=False, stop=True)
                            for hp in range(2):
                                nc.vector.tensor_copy(
                                    v_h[hp][:, tb, 0:64],
                                    ps[:, 128 * hp:128 * hp + 64])
                                nc.vector.tensor_copy(
                                    v_h[hp][:, tb, 65:129],
                                    ps[:, 128 * hp + 64:128 * hp + 128])
                    if stage == 'qkv':
                        dump_bf([qT[0][:, 0:R], qT[1][:, 0:R],
                                 kT[0][:, 0:R], kT[1][:, 0:R]])

                # ================= causal attention =================
                # 8-wide AllToAll per head-pair: slot j carries this core's
                # y (+ raw softmax denominators) for query quarter j%4,
                # duplicated to both batch halves; the receiver zeroes the
                # wrong-batch half via the zc input mask. Splitting by
                # head-pair overlaps a2a#0 with the second half of attention.
                cc_in = [dp.tile([8, 130, R], BF, name=f"cc_in{h}")
                         for h in range(2)]
                cc_out = [dp.tile([8, 130, R], BF, name=f"cc_out{h}")
                          for h in range(2)]
                with tc.tile_pool(name="pexp", bufs=4) as pep, \
                     tc.tile_pool(name="ysb", bufs=4) as yp, \
                     tc.tile_pool(name="aps", bufs=2, space="PSUM") as ap2:
                    for hp in (range(2) if want('attn') else []):
                        for qi in (3, 2, 1, 0):
                            nkb = 4 * qi + 4
                            qsl = slice(R * qi, R * (qi + 1))
                            o_ps0 = ap2.tile([65, R], FT, name="o_ps0",
                                             tag="opsum")
                            o_ps1 = ap2.tile([65, R], FT, name="o_ps1",
                                             tag="opsum")
                            # software-pipelined: AV of block kb-1 issues
                            # after the scores of block kb, so the PE never
                            # sits behind the in-flight exp in its queue.
                            p_prev = None
                            for kb in range(nkb):
                                ksl = slice(P * kb, P * (kb + 1))
                                s0 = ap2.tile([P, 2 * R], FT, name="s0",
                                              tag="spsum")
                                nc.tensor.matmul(s0[:, 0:R],
                                                 kT[hp][0:64, ksl],
                                                 qT[hp][0:64, qsl])
                                nc.tensor.matmul(s0[:, R:2 * R],
                                                 kT[hp][64:P, ksl],
                                                 qT[hp][64:P, qsl])
                                p0 = pep.tile([P, 2 * R], BF, name="p0",
                                              tag="pexp")
                                nc.scalar.activation(p0[:], s0[:], AF.Exp)
                                if kb >= 4 * qi:
                                    nc.vector.tensor_mul(
                                        p0[:], p0[:],
                                        mask_sb[:, kb - 4 * qi, :])
                                if p_prev is not None:
                                    nc.tensor.matmul(o_ps0[:],
                                                     v_h[hp][:, kb - 1, 0:65],
                                                     p_prev[:, 0:R],
                                                     start=(kb == 1),
                                                     stop=False)
                                    nc.tensor.matmul(
                                        o_ps1[:],
                                        v_h[hp][:, kb - 1, 65:130],
                                        p_prev[:, R:2 * R],
                                        start=(kb == 1), stop=False)
                                p_prev = p0
                            nc.tensor.matmul(o_ps0[:],
                                             v_h[hp][:, nkb - 1, 0:65],
                                             p_prev[:, 0:R],
                                             start=(nkb == 1), stop=True)
                            nc.tensor.matmul(o_ps1[:],
                                             v_h[hp][:, nkb - 1, 65:130],
                                             p_prev[:, R:2 * R],
                                             start=(nkb == 1), stop=True)
                            ystage = yp.tile([P, R], BF, name="ystage",
                                             tag="y")
                            nc.vector.tensor_copy(ystage[0:64, :],
                                                  o_ps0[0:64, :])
                            nc.vector.tensor_copy(ystage[64:P, :],
                                                  o_ps1[0:64, :])
                            d0 = yp.tile([1, R], BF, name="d0", tag="dt")
                            nc.vector.tensor_copy(d0[:], o_ps0[64:65, :])
                            d1 = yp.tile([1, R], BF, name="d1", tag="dt")
                            nc.vector.tensor_copy(d1[:], o_ps1[64:65, :])
                            for half in (0, 4):
                                sj = qi + half
                                nc.sync.dma_start(
                                    cc_in[hp][sj, 0:P, :], ystage[:])
                                nc.sync.dma_start(
                                    cc_in[hp][sj, P:P + 1, :], d0[:])
                                nc.sync.dma_start(
                                    cc_in[hp][sj, P + 1:P + 2, :], d1[:])
                        if want('attn'):
                            nc.gpsimd.collective_compute(
                                "AllToAll", OP.bypass, replica_groups=GRP8,
                                ins=[cc_in[hp][:]], outs=[cc_out[hp][:]])

            # ============ post-a2a: softmax scale + proj + residual ============
            # Two-round proj: the even head-pairs (delivered by a2a#0, which
            # completes during attention) are scaled and projected while
            # a2a#1 is still in flight; the odd half finishes afterwards.
            bp_ctx = tc.tile_pool(name="bpsum", bufs=4, space="PSUM")
            bp = bp_ctx.__enter__()
            with tc.tile_pool(name="ytp", bufs=1) as ytp, \
                 tc.tile_pool(name="wpp", bufs=1) as wpp:
                yt = [None] * CT

                def prep_dens(hp):
                    # raw denominators of head-pair parity hp from each
                    # batch half -> batched reciprocal; the wrong half is
                    # zeroed by the per-core zc mask so its y contribution
                    # vanishes in the combine below. Rows of the other
                    # parity stay at 1.0 (memset) and are never selected.
                    out = []
                    for half in range(2):
                        dr = ytp.tile([16, R], BF, name=f"draw{hp}{half}",
                                      tag="draw", bufs=4)
                        nc.vector.memset(dr[:], 1.0)
                        for src in range(4):
                            nc.sync.dma_start(
                                dr[4 * src + 2 * hp:4 * src + 2 * hp + 2, :],
                                cc_out[hp][4 * half + src, P:P + 2, :])
                        rd = ytp.tile([16, R], BF, name=f"rden{hp}{half}",
                                      tag="rden", bufs=4)
                        nc.vector.reciprocal(rd[:], dr[:])
                        nc.scalar.activation(rd[:], rd[:], AF.Identity,
                                             scale=zc_sb[:, half:half + 1])
                        out.append(rd)
                    return out

                def prep_yt(kt, dens):
                    ca = ytp.tile([P, R], BF, name=f"ca{kt}", tag="cand",
                                  bufs=3)
                    nc.sync.dma_start(ca[:], cc_out[kt % 2][kt // 2, 0:P, :])
                    cb = ytp.tile([P, R], BF, name=f"cb{kt}", tag="cand",
                                  bufs=3)
                    nc.sync.dma_start(cb[:],
                                      cc_out[kt % 2][4 + kt // 2, 0:P, :])
                    rdba = bp.tile([P, R], FT, name="rdba", tag="fc")
                    nc.tensor.matmul(rdba[:], sel16_sb[:, kt, :], dens[0][:])
                    nc.vector.tensor_mul(ca[:], ca[:], rdba[:])
                    rdbb = bp.tile([P, R], FT, name="rdbb", tag="fc")
                    nc.tensor.matmul(rdbb[:], sel16_sb[:, kt, :], dens[1][:])
                    nc.vector.tensor_mul(cb[:], cb[:], rdbb[:])
                    t = ytp.tile([P, R], BF, name=f"yt{kt}")
                    nc.vector.tensor_add(t[:], ca[:], cb[:])
                    yt[kt] = t

                if want('attn'):
                    wp_all = []
                    for mt in range(CT):
                        wt = wpp.tile([P, CT, P], BF, name=f"wp_t{mt}")
                        nc.gpsimd.dma_start(wt[:], d_wproj[mt])
                        wp_all.append(wt)
                    densE = prep_dens(0)
                    for kt in (0, 2, 4, 6):
                        prep_yt(kt, densE)
                    accE = []
                    if want('proj'):
                        for mt in range(CT):
                            psA = bp.tile([P, R], FT, name="ps_prA",
                                          tag="fc")
                            for j, kt in enumerate((0, 2, 4, 6)):
                                nc.tensor.matmul(psA[:], wp_all[mt][:, kt, :],
                                                 yt[kt][:], start=(j == 0),
                                                 stop=(j == 3))
                            acc = ytp.tile([P, R], BF, name=f"accE{mt}",
                                           tag="acc", bufs=CT)
                            nc.scalar.copy(acc[:], psA[:])
                            accE.append(acc)
                    densO = prep_dens(1)
                    for kt in (1, 3, 5, 7):
                        prep_yt(kt, densO)
                if stage == 'attn':
                    dump_bf([t[:] for t in yt])

                for mt in (range(CT) if want('proj') else []):
                    psB = bp.tile([P, R], FT, name="ps_prB", tag="fc")
                    for j, kt in enumerate((1, 3, 5, 7)):
                        nc.tensor.matmul(psB[:], wp_all[mt][:, kt, :],
                                         yt[kt][:], start=(j == 0),
                                         stop=False)
                    # fold the even-half partial back in on the PE
                    nc.tensor.matmul(psB[:], ident_sb[:], accE[mt][:],
                                     start=False, stop=True)
                    nc.vector.scalar_tensor_tensor(
                        X[mt][:], psB[:], bproj_sb[:, mt:mt + 1], X[mt][:],
                        op0=OP.add, op1=OP.add)

            # ================== adapters + MLP (token-local) ==================
            def layer_norm(dst_pool, g_sb, b_sb, tag):
                s_ps = pp.tile([1, R], FT, name=f"s_{tag}", tag="mm")
                q_ps = pp.tile([1, R], FT, name=f"q_{tag}", tag="mm")
                for ct in range(CT):
                    xbt = sqp.tile([P, R], BF, name=f"xb_{tag}", tag="xb",
                                   bufs=CT)
                    nc.scalar.copy(xbt[:], X[ct][:])
                    xsq = sqp.tile([P, R], BF, name=f"xsq_{tag}", tag="xsq")
                    nc.scalar.activation(xsq[:], X[ct][:], AF.Square)
                    nc.tensor.matmul(s_ps[:], ones[:], xbt[:],
                                     start=(ct == 0), stop=(ct == CT - 1))
                    nc.tensor.matmul(q_ps[:], ones[:], xsq[:],
                                     start=(ct == 0), stop=(ct == CT - 1))
                mu = lp.tile([1, R], BF, name=f"mu_{tag}", tag="ln")
                nc.scalar.mul(mu[:], s_ps[:], 1.0 / C)
                ex2 = lp.tile([1, R], FT, name=f"ex2_{tag}", tag="ln")
                nc.scalar.mul(ex2[:], q_ps[:], 1.0 / C)
                var = lp.tile([1, R], FT, name=f"var_{tag}", tag="ln")
                nc.vector.tensor_mul(var[:], mu[:], mu[:])
                nc.vector.tensor_sub(var[:], ex2[:], var[:])
                std = lp.tile([1, R], FT, name=f"std_{tag}", tag="ln")
                nc.scalar.activation(std[:], var[:], AF.Sqrt, bias=eps_sb[:])
                rstd = lp.tile([1, R], BF, name=f"rstd_{tag}", tag="ln")
                nc.vector.reciprocal(rstd[:], std[:])
                mu_b = pp.tile([P, R], FT, name=f"mub_{tag}", tag="mm")
                nc.tensor.matmul(mu_b[:], onesr[:], mu[:])
                rstd_b = pp.tile([P, R], FT, name=f"rsb_{tag}", tag="mm")
                nc.tensor.matmul(rstd_b[:], onesr[:], rstd[:])
                mub_sb = lp.tile([P, R], BF, name=f"mubs_{tag}", tag="lnb",
                                 bufs=4)
                nc.scalar.copy(mub_sb[:], mu_b[:])
                rsb_sb = lp.tile([P, R], BF, name=f"rsbs_{tag}", tag="lnb",
                                 bufs=4)
                nc.scalar.copy(rsb_sb[:], rstd_b[:])
                out_tiles = []
                for ct in range(CT):
                    xn = dst_pool.tile([P, R], BF, name=f"{tag}_{ct}",
                                       tag="xln")
                    nc.vector.tensor_sub(xn[:], X[ct][:], mub_sb[:])
                    nc.vector.tensor_mul(xn[:], xn[:], rsb_sb[:])
                    nc.scalar.activation(xn[:], xn[:], AF.Identity,
                                         scale=g_sb[:, ct:ct + 1],
                                         bias=b_sb[:, ct:ct + 1])
                    out_tiles.append(xn)
                return out_tiles

            def adapter(d_dw, d_uw, bd_sb, bu_sb, tag):
                with tc.tile_pool(name=f"ad_{tag}", bufs=1) as adp:
                    ad = adp.tile([P, CT, M], BF, name=f"ad_{tag}")
                    nc.sync.dma_start(ad[:], d_dw[:])
                    au = adp.tile([M, CT, P], BF, name=f"au_{tag}")
                    nc.sync.dma_start(au[:], d_uw[:])
                    ps_a = bp.tile([M, R], FT, name=f"psa_{tag}", tag="fc")
                    for kt in range(CT):
                        xbt = adp.tile([P, R], BF, name=f"xb_{tag}", tag="xb",
                                       bufs=3)
                        nc.scalar.copy(xbt[:], X[kt][:])
                        nc.tensor.matmul(ps_a[:], ad[:, kt, :], xbt[:],
                                         start=(kt == 0), stop=(kt == CT - 1))
                    ar = adp.tile([M, R], BF, name=f"ar_{tag}")
                    nc.scalar.activation(ar[:], ps_a[:], AF.Relu,
                                         bias=bd_sb[:, 0:1])
                    for mt in range(CT):
                        ps = bp.tile([P, R], FT, name=f"psu_{tag}", tag="fc")
                        nc.tensor.matmul(ps[:], au[:, mt, :], ar[:])
                        nc.vector.scalar_tensor_tensor(
                            X[mt][:], ps[:], bu_sb[:, mt:mt + 1], X[mt][:],
                            op0=OP.add, op1=OP.add)

            if want('a1'):
                adapter(d_a1d, d_a1u, b1d_sb, b1u_sb, "a1")

            # ---------------- LN2 + MLP ----------------
            with tc.tile_pool(name="xln2", bufs=CT) as x2p, \
                 tc.tile_pool(name="wfcp", bufs=3) as wfp, \
                 tc.tile_pool(name="hT", bufs=32) as hp_, \
                 tc.tile_pool(name="wmpp", bufs=2) as wmp:

                x2 = (layer_norm(x2p, ln2g_sb, ln2b_sb, "ln2")
                      if want('mlp') else [])
                hT = []
                for mt in (range(32) if want('mlp') else []):
                    wt = wfp.tile([P, CT, P], BF, name="wfc_t", tag="wfc")
                    nc.sync.dma_start(wt[:], d_wfc[mt])
                    ps = bp.tile([P, R], FT, name="ps_fc", tag="fc")
                    for kt in range(CT):
                        nc.tensor.matmul(ps[:], wt[:, kt, :], x2[kt][:],
                                         start=(kt == 0), stop=(kt == CT - 1))
                    ht = hp_.tile([P, R], BF, name="ht", tag="hT")
                    nc.scalar.activation(ht[:], ps[:], AF.Gelu_apprx_tanh,
                                         bias=bfc_sb[:, mt:mt + 1])
                    hT.append(ht)
                for mt in (range(CT) if want('mlp') else []):
                    wt = wmp.tile([P, 32, P], BF, name="wmp_t", tag="wmp")
                    nc.sync.dma_start(wt[:], d_wmp[mt])
                    ps = bp.tile([P, R], FT, name="ps_mp", tag="fc")
                    for kt in range(32):
                        nc.tensor.matmul(ps[:], wt[:, kt, :], hT[kt][:],
                                         start=(kt == 0), stop=(kt == 31))
                    nc.vector.scalar_tensor_tensor(
                        X[mt][:], ps[:], bmp_sb[:, mt:mt + 1], X[mt][:],
                        op0=OP.add, op1=OP.add)

            if want('full'):
                adapter(d_a2d, d_a2u, b2d_sb, b2u_sb, "a2")

            bp_ctx.__exit__(None, None, None)

            # ---------------- output ----------------
            if stage in ('x', 'proj', 'a1', 'mlp', 'full'):
                for ct in range(CT):
                    nc.sync.dma_start(d_out[ct], X[ct][:])

    nc.compile()
    return nc


def _lhst_tiles(w, nmt):
    # w [K, Mout] -> [nmt, P, K//P, P]: tile[mt, p, kt, m] = w[P*kt+p, P*mt+m]
    kk, mm = w.shape
    return np.ascontiguousarray(
        w.reshape(kk // P, P, nmt, P).transpose(2, 1, 0, 3))


def _col_vec(v, nmt):
    return np.ascontiguousarray(v.reshape(nmt, P).T)


def _prep_shared(inputs):
    """Host-side tiling of the head-group-independent weights/biases."""
    f32 = np.float32
    bf16 = ml_dtypes.bfloat16
    W = {k: np.ascontiguousarray(np.asarray(v, dtype=f32))
         for k, v in inputs.items()}

    shared = {
        'wproj': _lhst_tiles(W['proj_w'], CT),
        'wfc': _lhst_tiles(W['fc_w'], 32),
        'wmp': _lhst_tiles(W['mlp_pw'], CT),
        'a1d': np.ascontiguousarray(
            W['a1_dw'].reshape(CT, P, M).transpose(1, 0, 2)),
        'a1u': np.ascontiguousarray(W['a1_uw'].reshape(M, CT, P)),
        'a2d': np.ascontiguousarray(
            W['a2_dw'].reshape(CT, P, M).transpose(1, 0, 2)),
        'a2u': np.ascontiguousarray(W['a2_uw'].reshape(M, CT, P)),
        'bproj': _col_vec(W['proj_b'], CT),
        'bfc': _col_vec(W['fc_b'], 32),
        'bmp': _col_vec(W['mlp_pb'], CT),
        'b1d': np.ascontiguousarray(W['a1_db'].reshape(M, 1)),
        'b1u': _col_vec(W['a1_ub'], CT),
        'b2d': np.ascontiguousarray(W['a2_db'].reshape(M, 1)),
        'b2u': _col_vec(W['a2_ub'], CT),
        'ln1g': _col_vec(W['ln1_g'], CT),
        'ln1b': _col_vec(W['ln1_b'], CT),
        'ln2g': _col_vec(W['ln2_g'], CT),
        'ln2b': _col_vec(W['ln2_b'], CT),
    }
    shared['onesc'] = np.ones((P, 1), dtype=f32)
    shared['ident'] = np.eye(P, dtype=f32)
    shared['onesr'] = np.ones((1, P), dtype=f32)
    shared['vones'] = np.ones((P, 16), dtype=f32)
    sel16 = np.zeros((CT, 16, P), dtype=f32)
    for hp in range(CT):
        sel16[hp, 2 * hp, 0:64] = 1.0
        sel16[hp, 2 * hp + 1, 64:P] = 1.0
    shared['sel16'] = sel16
    # causal masks for the 4 diagonal key blocks of any query quarter,
    # duplicated along the free axis for the two packed heads
    kj = np.arange(P)[:, None]
    qi = np.arange(R)[None, :]
    mask = np.zeros((P, 4, 2 * R), dtype=f32)
    for j in range(4):
        m = ((P * j + kj) <= qi).astype(f32)
        mask[:, j, 0:R] = m
        mask[:, j, R:2 * R] = m
    shared['mask'] = mask
    for k in ('wproj', 'wfc', 'wmp', 'a1d', 'a1u', 'a2d', 'a2u',
              'onesc', 'onesr', 'vones', 'sel16', 'mask', 'ident'):
        shared[k] = np.ascontiguousarray(shared[k].astype(bf16))
    return shared


def _prep_head(inputs, g):
    """Per-head-group (g = 0..3, heads 4g..4g+3) QKV weight slices."""
    f32 = np.float32
    bf16 = ml_dtypes.bfloat16
    aw = np.asarray(inputs['attn_w'], dtype=f32)
    ab = np.asarray(inputs['attn_b'], dtype=f32)
    s = f32(1.0 / np.sqrt(HD))
    cs = slice(256 * g, 256 * (g + 1))
    wq = aw[:, 0:C][:, cs] * s
    wk = aw[:, C:2 * C][:, cs]
    wv = aw[:, 2 * C:3 * C][:, cs]
    bq = ab[0:C][cs] * s
    bk = ab[C:2 * C][cs]
    bv = ab[2 * C:3 * C][cs]
    wqk = np.concatenate([_lhst_tiles(wq, 2), _lhst_tiles(wk, 2)], axis=0)
    wv_m = np.ascontiguousarray(
        wv.reshape(CT, P, 256).transpose(1, 0, 2))
    return {
        'wqk': np.ascontiguousarray(wqk.astype(bf16)),
        'wv': np.ascontiguousarray(wv_m.astype(bf16)),
        'bqk': np.ascontiguousarray(
            np.concatenate([bq, bk]).reshape(4, P).T),
        'bv': np.ascontiguousarray(bv.reshape(1, 256).astype(bf16)),
    }


def _prep_core(x, c):
    b, g = c // 4, c % 4
    xb_ = np.asarray(x[b], dtype=np.float32)                  # [T, C]
    xf = np.ascontiguousarray(
        xb_.T.reshape(CT, P, T).transpose(1, 0, 2)).astype(
            ml_dtypes.bfloat16)                               # [P, CT, T]
    xo = np.asarray(x[b, R * g:R * (g + 1), :], dtype=np.float32)
    xT = np.ascontiguousarray(
        xo.T.reshape(CT, P, R).transpose(1, 0, 2))            # [P, CT, R]
    zc = np.zeros((16, 2), dtype=np.float32)
    zc[:, b] = 1.0
    return {'xf': np.ascontiguousarray(xf), 'xT': xT, 'zc': zc}


def _run(inputs, trace=False, stage='full'):
    if stage not in _CACHE:
        _CACHE[stage] = _build(stage)
    nc = _CACHE[stage]
    shared = _prep_shared(inputs)
    heads = [_prep_head(inputs, g) for g in range(4)]
    x = np.asarray(inputs['x'], dtype=np.float32)
    in_maps = []
    for c in range(NCORES):
        m = dict(shared)
        m.update(heads[c % 4])
        m.update(_prep_core(x, c))
        in_maps.append(m)
    kwargs = {}
    if trace:
        from trn_agent_boot.trn_boot import _ntff_profile_via_ctypes
        hook = _ntff_profile_via_ctypes('/opt/axon/libaxon_pjrt.so')
        mod = types.ModuleType('antenv.axon_hooks')
        mod.get_axon_ntff_profile_hook = lambda: hook
        sys.modules['antenv.axon_hooks'] = mod
        bass_utils.upload_artifacts = lambda tmpdir: "/tmp/no-upload"
        kwargs['trace'] = True
    res = bass_utils.run_bass_kernel_spmd(
        nc, in_maps, core_ids=list(range(NCORES)), **kwargs)
    y = np.zeros((B, T, C), dtype=np.float32)
    for c in range(NCORES):
        b, g = c // 4, c % 4
        o = res.results[c]['out']          # [CT, P, R]
        y[b, R * g:R * (g + 1), :] = o.reshape(C, R).T
    return y, res


def kernel(**inputs):
    y, _ = _run(inputs, trace=False)
    return y
